# revision 61
# baseline (speedup 1.0000x reference)
"""BQuantConv1d Trainium2 kernel.

Math: the reference's per-token LUT + gather is algebraically a matmul:
  out[n, f] = sum_i x[n, i] * W[i, f] + bias[f]
  W[8g+j, f] = sum_b scale[b, f] * (2*bit_{7-j}(binary[b, g, f]) - 1)

Sharding: 2 token-groups x 4 f-groups over 8 cores, no collectives
(host slices inputs / concatenates outputs; layout-only host work).
Contraction order is permuted to i' = j*128 + g (host permutes xT rows to
match) so each decoded weight chunk j lands on contiguous partitions.

Per core:
  - decode W'(1024, 256) from int16 codes with a sign-bit trick:
    W element = +-scale[b, f] exactly, built by XORing the fp16 scale's
    sign bit (scales arrive sign-pre-flipped) with the masked quant bit
    (c << (8+j)) & 0x8000, as int32 SWAR on DVE (bitvec ops are DVE-only
    and 32-bit-only on walrus); the 8-way b-reduction is an fp16 add
    tree with the first level (h1) on DVE and the h2/w levels offloaded
    to the otherwise-idle GPSIMD engine.  Chunks 0/1 run TS/xor in
    b-halves gated on half-sized cd/sc DMAs (starts the decode ~0.8us
    earlier); each later chunk's TS is emitted between xor_j and h1_j so
    it fills the write-ack window and the greedy scheduler doesn't park
    the next 1.1us xor in front of h1_j; chunk 7's tree stays on DVE
    (GPSIMD latency would gate the tail) split by f-half so the fb=0
    matmuls start while fb=1 is still reducing;
  - outT[f_shard, n_shard] = W'.T @ xT on the PE in fp16, accumulating
    the 8 contraction chunks across 8 concurrent PSUM banks (f32);
    each bank is seeded with the bias via a K=1 bias x ones matmul --
    the seeds run in the pipeline head while the PE is otherwise idle
    and double as its p-state warmup;
  - PSUM pairs evacuated as fp16 (copies alternating DVE/ACT) into
    double-wide tiles; the out DRAM layout is token-major per f-block,
    so each ch-pair ships as ONE partition-major DMA (4 output DMAs
    instead of 8 -- the issue+HWDGE pipe, not bytes, dominates the
    tail), on the SP queue except one mid-stream pair on GPSIMD SWDGE.

Cost-model notes (TimelineSim, the graded metric): DVE is the
bottleneck engine (~19us busy: TS 8B/cyc, TT-xor 4B/cyc, fp16 adds
4B/cyc); total = DVE-end + ~8us of structural head/tail (2us DMA issue
pipe + 0.9us DMA-completion sem props on the head; 16-matmul PE drain +
evac chains + serialized output DMA transfers + 0.9us sem on the tail).
"""

import numpy as np

try:
    import concourse.bass as bass  # noqa: F401
except ImportError:
    import sys

    sys.path.insert(0, "/opt/trn_rl_repo")
    import concourse.bass as bass  # noqa: F401

import concourse.bacc as bacc
import concourse.mybir as mybir
import concourse.tile as tile

B, T, NX, NF = 2, 2048, 1024, 1024
N_TOK = B * T
BITS = 8
G = NX // 8  # 128 code groups
PT, PF = 2, 4  # token-parallel x feature-parallel
TOK = N_TOK // PT  # tokens per core
NFS = NF // PF  # output features per core
P = 128
MM_N = 512  # moving free dim per matmul

AX = mybir.AxisListType
OP = mybir.AluOpType
F32 = mybir.dt.float32
BF16 = mybir.dt.float16  # compute dtype (fp16: same SWAR, more mantissa)
I16 = mybir.dt.int16
I32 = mybir.dt.int32
ACT_F = mybir.ActivationFunctionType
BF16NP = np.float16

MSK = -2147450880  # 0x80008000 as int32


def build_graph(nc, tok=TOK, nfs=NFS):
    nfb = nfs // P  # f blocks of 128 (2)
    nch = tok // MM_N  # moving chunks (4)
    xt_d = nc.dram_tensor("xt", (8, P, tok), BF16, kind="ExternalInput")
    cd_d = nc.dram_tensor("codes", (P, 8 * nfs), I16, kind="ExternalInput")
    sc_d = nc.dram_tensor("scales", (P, 8 * nfs), BF16, kind="ExternalInput")
    bi_d = nc.dram_tensor("biasr", (1, nfs), BF16, kind="ExternalInput")
    out_d = nc.dram_tensor("out", (nfb, P, tok), BF16, kind="ExternalOutput")

    with tile.TileContext(nc) as tc:
        with (
            tc.tile_pool(name="xp", bufs=8) as xp,
            tc.tile_pool(name="cp", bufs=8) as cp,
            tc.tile_pool(name="wp", bufs=8) as wp,
            tc.tile_pool(name="qp", bufs=6) as qp,
            tc.tile_pool(name="cst", bufs=1) as cst,
            tc.tile_pool(name="op", bufs=8) as op_,
            tc.tile_pool(name="pp", bufs=8, space="PSUM") as pp,
        ):
            # --- loads; codes/scales first, halves interleaved: chunk 0's
            # decode runs in b-halves gated on each half-DMA, cutting ~1.2us
            # off the whole decode stream's start ---
            H = 4 * nfs
            cd = cp.tile([P, 8 * nfs], I16, tag="cd")
            sc_bc = cst.tile([P, 8 * nfs], BF16, tag="sc_bc")
            nc.sync.dma_start(cd[:, :H], cd_d[:, :H])
            nc.sync.dma_start(sc_bc[:, :H], sc_d[:, :H])
            nc.sync.dma_start(cd[:, H:], cd_d[:, H:])
            nc.sync.dma_start(sc_bc[:, H:], sc_d[:, H:])
            bi_row = cst.tile([1, nfs], BF16, tag="bi_row")
            nc.sync.dma_start(bi_row[:], bi_d[:])
            ones = cst.tile([1, MM_N], BF16, tag="ones")
            nc.gpsimd.memset(ones[:], 1.0)
            xts = []
            for j in range(8):
                xt = xp.tile([P, tok], BF16, tag="xt")
                nc.sync.dma_start(xt[:], xt_d[j])
                xts.append(xt)

            # --- bias seeds: PSUM[fb,ch] := bias via K=1 matmul ---
            # They run in the pipeline head (PE idle until chunk 0's W is
            # decoded) and double as the PE p-state warmup.
            pss = {}
            for fb in range(nfb):
                for ch in range(nch):
                    ps = pp.tile([P, MM_N], F32, tag="ps", name=f"ps{fb}_{ch}")
                    pss[(fb, ch)] = ps
                    nc.tensor.matmul(
                        ps[:],
                        bi_row[:, fb * P : (fb + 1) * P],
                        ones[:],
                        start=True,
                        stop=False,
                    )

            # --- decode W chunks ---
            # Sign-bit trick: masked quant bit (inverted) XORed onto the
            # fp16 scale's sign gives +-scale exactly.  Bitvec ops are
            # DVE-only and 32-bit-only on walrus, so they run as int32 SWAR
            # over int16-lane pairs: a left shift by 8+j sources each
            # lane's bit 15 from within the same lane, and the 0x80008000
            # mask keeps only the two sign bits.  The bit inversion is
            # folded into a one-time sign-flip of the scale tile:
            #   ((c << (8+j)) & M) ^ (sc ^ M)  ==  ((~c << (8+j)) & M) ^ sc
            def emit_ts(j):
                sg = qp.tile([P, 8 * nfs], I16, tag="sg", name=f"sg{j}")
                nc.vector.tensor_scalar(
                    sg[:].bitcast(I32), cd[:].bitcast(I32), 8 + j, MSK,
                    OP.logical_shift_left, OP.bitwise_and,
                )
                return sg

            def emit_xor(j, sg):
                wsg = qp.tile([P, 8 * nfs], I16, tag="wsg", name=f"wsg{j}")
                nc.vector.tensor_tensor(
                    wsg[:].bitcast(I32), sg[:].bitcast(I32),
                    sc_bc[:].bitcast(I32), OP.bitwise_xor,
                )
                return wsg

            def emit_h1(j, wsg):
                wv = wsg[:].bitcast(BF16)
                h1 = qp.tile([P, 4 * nfs], BF16, tag="h1", name=f"h1_{j}")
                nc.vector.tensor_tensor(
                    h1[:], wv[:, : 4 * nfs], wv[:, 4 * nfs :], OP.add
                )
                return h1

            def tree_tail(j, h1, teng):
                h2 = qp.tile([P, 2 * nfs], BF16, tag="h2", name=f"h2_{j}")
                teng.tensor_tensor(
                    h2[:], h1[:, : 2 * nfs], h1[:, 2 * nfs :], OP.add
                )
                w = wp.tile([P, nfs], BF16, tag="w", name=f"w{j}")
                teng.tensor_tensor(w[:], h2[:, :nfs], h2[:, nfs:], OP.add)
                return w

            # b-reduction fp16 add tree: h1 on DVE; h2/w on GPSIMD for
            # chunks 0-5 (chunk 0's on DVE: pipeline head).  The next
            # chunk's TS is emitted BETWEEN xor_j and h1_j: it is always
            # ready, so it fills the write-ack window after xor_j and the
            # scheduler then runs h1_j instead of parking the next 1.1us
            # xor in front of it.  Chunks 6/7 keep their trees on DVE
            # (GPSIMD's serial latency would bunch w6 against w7 and double
            # the PE drain); chunk 7's h2/w are split by f-half so the fb=0
            # matmuls start while fb=1 is still reducing.
            ws = {}
            # chunks 0/1 in b-halves so each half starts on its half-DMA
            # (fills the DVE while the second cd/sc halves are in flight)
            sgs, wsgs = {}, {}
            for j in (0, 1):
                sg = qp.tile([P, 8 * nfs], I16, tag="sg", name=f"sg{j}")
                wsg = qp.tile([P, 8 * nfs], I16, tag="wsg", name=f"wsg{j}")
                for half in range(2):
                    lo, hi = half * H, (half + 1) * H
                    nc.vector.tensor_scalar(
                        sg[:, lo:hi].bitcast(I32), cd[:, lo:hi].bitcast(I32),
                        8 + j, MSK, OP.logical_shift_left, OP.bitwise_and,
                    )
                    nc.vector.tensor_tensor(
                        wsg[:, lo:hi].bitcast(I32),
                        sg[:, lo:hi].bitcast(I32),
                        sc_bc[:, lo:hi].bitcast(I32), OP.bitwise_xor,
                    )
                sgs[j], wsgs[j] = sg, wsg
            for j in range(1, 8):
                if j > 1:
                    sgs[j] = emit_ts(j)
                h1 = emit_h1(j - 1, wsgs[j - 1])
                ws[j - 1] = tree_tail(j - 1, h1, nc.gpsimd)
                if j > 1:
                    wsgs[j] = emit_xor(j, sgs[j])
            h1_7 = emit_h1(7, wsgs[7])
            h1v = h1_7[:].rearrange("p (b f) -> p b f", b=4)
            w7 = wp.tile([P, nfs], BF16, tag="w", name="w7")
            for half in range(2):
                f0, f1 = half * P, (half + 1) * P
                h2h = qp.tile([P, 2, P], BF16, tag="h2h", name=f"h2h{half}")
                nc.vector.tensor_tensor(
                    h2h[:], h1v[:, :2, f0:f1], h1v[:, 2:, f0:f1], OP.add
                )
                nc.vector.tensor_tensor(
                    w7[:, f0:f1], h2h[:, 0], h2h[:, 1], OP.add
                )
            ws[7] = w7

            # --- matmul: outT[f, n] = bias + sum_j W_j.T @ xT_j ---
            # j outermost: each W chunk feeds the PE as soon as it is
            # decoded, all nfb*nch PSUM banks accumulate concurrently.
            # The last chunk is issued bank-by-bank so evacuation and
            # output DMA overlap the remaining j=7 matmuls.
            for j in range(7):
                for fb in range(nfb):
                    for ch in range(nch):
                        nc.tensor.matmul(
                            pss[(fb, ch)][:],
                            ws[j][:, fb * P : (fb + 1) * P],
                            xts[j][:, ch * MM_N : (ch + 1) * MM_N],
                            start=False,
                            stop=False,
                        )
            # evacuation alternates DVE/ACT (GPSIMD cannot read PSUM) into a
            # double-wide tile per ch-pair; adjacent ch tiles are contiguous
            # in DRAM, so each pair ships as ONE output DMA.  4 DMAs instead
            # of 8 halves the issue+HWDGE serialization in the tail, and
            # putting them on the idle SP/GPSIMD queues keeps both evac
            # engines' sequencers free of cross-engine DMA waits.
            k = 0
            for fb in range(nfb):
                for cp2 in range(nch // 2):
                    obw = op_.tile([P, 2 * MM_N], BF16, tag="obw",
                                   name=f"obw{fb}_{cp2}")
                    for half in range(2):
                        ch = 2 * cp2 + half
                        nc.tensor.matmul(
                            pss[(fb, ch)][:],
                            ws[7][:, fb * P : (fb + 1) * P],
                            xts[7][:, ch * MM_N : (ch + 1) * MM_N],
                            start=False,
                            stop=True,
                        )
                        dst = obw[:, half * MM_N : (half + 1) * MM_N]
                        if k % 2 == 0:
                            nc.vector.tensor_copy(dst, pss[(fb, ch)][:])
                        else:
                            nc.scalar.copy(dst, pss[(fb, ch)][:])
                        k += 1
                    # pair (2,3) on GPSIMD (its ~2us SWDGE pipe hides mid
                    # stream), the rest on SP whose sem-gated issues overlap;
                    # out DRAM is token-major per fb so the ch-pair is a
                    # plain partition-major slice (no AP transpose)
                    deng = nc.gpsimd if (fb, cp2) == (1, 0) else nc.sync
                    deng.dma_start(
                        out_d[fb][:, 2 * cp2 * MM_N : (2 * cp2 + 2) * MM_N],
                        obw[:],
                    )
    nc.compile()
    return nc


_I_PERM = 8 * (np.arange(NX) % G) + np.arange(NX) // G  # i' -> i


def host_prep(x, binary, scale, bias):
    """Layout-only sharding (plus x's fp16 compute-precision cast).
    Returns in_maps for cores 0..7 (pt = c//PF, pf = c%PF)."""
    x2 = np.ascontiguousarray(x.reshape(N_TOK, NX).T)[_I_PERM]  # (NX, N)
    x2 = x2.astype(BF16NP)  # compute dtype
    binary16 = binary.astype(np.int16)  # lossless: codes are 0..255
    in_maps = []
    for c in range(8):
        pt, pf = c // PF, c % PF
        f0 = pf * NFS
        xs = np.ascontiguousarray(x2[:, pt * TOK : (pt + 1) * TOK]).reshape(
            8, P, TOK
        )
        cs = np.ascontiguousarray(
            binary16[:, :, f0 : f0 + NFS].transpose(1, 0, 2)
        ).reshape(P, 8 * NFS)
        ss = np.ascontiguousarray(
            np.broadcast_to(
                (-scale[:, f0 : f0 + NFS].astype(BF16NP)).reshape(1, 8 * NFS),
                (P, 8 * NFS),
            )
        )
        bs = np.ascontiguousarray(
            bias[f0 : f0 + NFS].astype(BF16NP).reshape(1, NFS)
        )
        in_maps.append({"xt": xs, "codes": cs, "scales": ss, "biasr": bs})
    return in_maps


def host_assemble(results):
    """results[c]["out"]: (NFB, 128, TOK) -> full (B, T, NF)."""
    outT = np.empty((NF, N_TOK), dtype=np.float32)
    for c in range(8):
        pt, pf = c // PF, c % PF
        o = np.asarray(results[c]["out"], dtype=np.float32).reshape(NFS, TOK)
        outT[pf * NFS : (pf + 1) * NFS, pt * TOK : (pt + 1) * TOK] = o
    return np.ascontiguousarray(outT.T).reshape(B, T, NF)


_NC_CACHE = {}


def _get_nc():
    if "nc" not in _NC_CACHE:
        nc = bacc.Bacc(None, target_bir_lowering=False)
        build_graph(nc)
        _NC_CACHE["nc"] = nc
    return _NC_CACHE["nc"]


def kernel(**inputs):
    from concourse.bass_utils import run_bass_kernel_spmd

    inputs = {k: np.asarray(v) for k, v in inputs.items()}
    in_maps = host_prep(
        inputs["x"], inputs["binary"], inputs["scale"], inputs["bias"]
    )
    res = run_bass_kernel_spmd(_get_nc(), in_maps, core_ids=list(range(8)))
    return host_assemble(res.results)
